# revision 73
# baseline (speedup 1.0000x reference)
"""Causal multi-head attention block on 8 Trainium2 NeuronCores.

Problem: x[4,2048,1024] -> qkv proj -> 16-head causal attention -> out proj.

Sharding: 8 cores = 4 batches x 2 head-groups (8 heads each). Each core
computes, for its (batch, head-group):
  - qT/kT (feature-on-partition, via PE-transposed x) and v (natural layout)
  - causal attention with scores computed transposed (scoresT[j, i]):
    softmax without max-subtraction (scores are O(1) for these inputs),
    row-sums from an appended ones-column on v in the attn@v matmul
  - partial out-projection with its 512 rows of W_proj
Host sums the two partials per batch and adds b_proj.

Perf structure — one fused PE stream (v3):
  - all matmul operands bfloat16 (1 c/r at any N, ~half the PE power of
    f32r so the GPIO power throttle stays quiet); PSUM accumulation f32
  - the attention phase is ACT-bound per j-tile (exp ~1.28us vs ~0.85us
    of PE work), so the qkv projection and out-projection are NOT separate
    phases: they are a queue of small "filler" bursts (transpose group /
    v-tile / qk-tile / proj-half) popped between attention j-tiles.  The
    PE therefore never stalls on exp, which also keeps the PE DVFS ramp
    at its 2.4 GHz pstate (isolated matmul bursts run at 1.2 GHz).
  - qkv for s-block sb is produced as filler during attention i-block
    sb-1; out-proj for i-block ib runs as filler after its softmax
    normalization completes (during i-block ib+1)
  - scores sc pool double-buffered; emission order sc(jt) -> exp(jt) ->
    mask(jt) -> av(jt-1) software-pipelines the exp latency
  - exp on ACT only; causal mask via GpSimd affine_select on the
    diagonal 128-col block; diagonal blocks fully N-trimmed (bf16)
  - softmax reciprocal on a [64,8]-spread layout (DRAM bounce) instead of
    a [1,512] single-partition op (3.3us -> ~50ns on DVE); the LAST
    i-block normalizes per-section (hidden under attention), and its
    final head-pair skips the DMA bounce entirely: reciprocal on the
    (drain-idle) ACT engine + a PE partition-broadcast matmul
  - PSUM budget exactly 8 banks: sc 2x2 + oa 2x1 + transpose 1 + shared
    qkv/proj accumulator 1
  - normalized output in per-i-block tiles (coarse dep tracking makes a
    shared tile's norm writes false barriers for filler reads); the 8
    norm muls spread over three section boundaries so the DVE queue never
    blocks the filler drains the PE needs next
Measured end-to-end rel err vs the fp64 reference: ~3.4e-3 (bf16).
HW exec ~294us (thermally noisy up to ~10%; f32r two-phase baseline was
~390-406us).
"""

import sys
import types as _types
from collections import deque

import numpy as np

import concourse.mybir as mybir
import concourse.tile as tile
from concourse import bacc
from concourse.bass import ts
from concourse.bass_utils import run_bass_kernel_spmd

# ---- problem constants (hardcoded per harness contract) ----
B, S, D, H = 4, 2048, 1024, 16
HD = D // H            # 64 head dim
HPC = H // 2           # 8 heads per core
FG = HPC * HD          # 512 features per head-group
NCORES = 8
NST = S // 128         # 16 s-tiles
NDT = D // 128         # 8 d-tiles
NSB = S // 512         # 4 s/i-blocks

F32 = mybir.dt.float32
MMD = mybir.dt.bfloat16
EXP = mybir.ActivationFunctionType.Exp


def _install_ntff_hook():
    """run_bass_kernel_spmd(trace=True) under axon needs antenv.axon_hooks,
    absent in this image; shim it with the boot module's ctypes hook."""
    if "antenv.axon_hooks" in sys.modules:
        return
    try:
        from trn_agent_boot.trn_boot import _ntff_profile_via_ctypes
    except ImportError:
        return
    m = _types.ModuleType("antenv.axon_hooks")
    m.get_axon_ntff_profile_hook = lambda: _ntff_profile_via_ctypes(
        "/opt/axon/libaxon_pjrt.so"
    )
    m.set_axon_ntff_profile_hook = lambda h: None
    sys.modules["antenv.axon_hooks"] = m


def _body(tc, io):
    nc = tc.nc
    x_r = io["x"].rearrange("(st p) d -> st p d", p=128)      # [16,128,1024]
    wq_r = io["wq"].rearrange("(dt p) f -> dt p f", p=128)    # [8,128,512]
    wk_r = io["wk"].rearrange("(dt p) f -> dt p f", p=128)
    wv_r = io["wv"].rearrange("(dt p) f -> dt p f", p=128)
    wp_r = io["wp"].rearrange("(ct p) e -> ct p e", p=128)    # [4,128,1024]
    out_r = io["out"].rearrange("(st p) e -> st p e", p=128)  # [16,128,1024]

    with tc.tile_pool(name="persist", bufs=1) as pp:
        qT = pp.tile([128, 4, S], MMD, name="qT")             # [f, pair, s]
        kT = pp.tile([128, 4, S], MMD, name="kT")
        vA = pp.tile([128, NST, HPC, HD + 1], MMD, name="vA")  # v | ones
        # normalized attention output, one tile PER i-block: dependency
        # tracking on a shared tile is coarse, so any norm write would act
        # as a false barrier for later-emitted out-proj reads of OTHER
        # blocks running as fillers
        outTb = [pp.tile([128, 4, 512], MMD, name=f"outTb{i}")
                 for i in range(NSB)]
        const = pp.tile([128, 128], MMD, name="const")        # identity
        wvt = pp.tile([128, NDT, 512], MMD, name="wvt")
        wqt = pp.tile([128, NDT, 512], MMD, name="wqt")
        wkt = pp.tile([128, NDT, 512], MMD, name="wkt")
        wpt = pp.tile([128, 4, 2, 512], MMD, name="wpt")
        # x^T ping-pong: A(sb) writes xTs[sb%2] while A(sb-1)'s last
        # consumers may still be in flight
        xTs = [pp.tile([128, NDT, 512], MMD, name=f"xT{i}") for i in range(2)]

        nc.sync.dma_start(out=const, in_=io["ident"])

        # broadcast-copy a 1.0 constant into the ones column
        ones1 = pp.tile([128, 1], F32, name="ones1")
        nc.vector.memset(ones1, 1.0)
        nc.vector.tensor_copy(
            vA[:, :, :, HD : HD + 1],
            ones1.unsqueeze(1).to_broadcast([128, NST, HPC, 1]),
        )
        # [1,64] ones row: stationary for the drain-time PE partition-
        # broadcast of the last head-pair's softmax reciprocals
        ones64 = pp.tile([1, HD], F32, name="ones64")
        nc.vector.tensor_copy(
            ones64, ones1[0:1, 0:1].to_broadcast([1, HD]))
        # scratch operand for the PE clock pre-ramp (see below)
        warm = pp.tile([128, 512], MMD, name="warm")
        nc.vector.memset(warm, 1.0)

        with (
            tc.tile_pool(name="pxn", bufs=4) as pxn,
            tc.tile_pool(name="pfill", bufs=1, space="PSUM") as pfill,
            tc.tile_pool(name="psc", bufs=2, space="PSUM") as psc,
            tc.tile_pool(name="poa", bufs=1, space="PSUM") as poa,
            tc.tile_pool(name="pat", bufs=4) as pat,
            tc.tile_pool(name="p2n", bufs=2) as p2n,
            tc.tile_pool(name="p2_dr", bufs=4, space="DRAM") as p2d,
            tc.tile_pool(name="p3_r", bufs=4) as p3s,
        ):
            # x(sb=0) prefetch FIRST on gpsimd — the first transposes need
            # it; v/q weights stream on sync in parallel, k weights behind
            # the prefetch on gpsimd
            xn0s = []
            for st4 in range(4):
                xn = pxn.tile([128, D], MMD, name="xn", tag="xn", bufs=4)
                nc.gpsimd.dma_start(out=xn, in_=x_r[st4])
                xn0s.append(xn)
            for dt_ in range(NDT):
                nc.sync.dma_start(out=wvt[:, dt_, :], in_=wv_r[dt_])
                nc.gpsimd.dma_start(out=wkt[:, dt_, :], in_=wk_r[dt_])
            for dt_ in range(NDT):
                nc.sync.dma_start(out=wqt[:, dt_, :], in_=wq_r[dt_])
            for ct in range(4):
                for et in range(2):
                    nc.sync.dma_start(out=wpt[:, ct, et, :],
                                      in_=wp_r[ct][:, ts(et, 512)])

            # PE clock pre-ramp: the DVFS pstate needs ~3us of continuous
            # activity to leave the 0.65/1.2 GHz states; the PE is
            # otherwise idle until the first x tile lands (~11us).  Dummy
            # matmuls on the memset scratch (results never read) keep the
            # array busy through that window so the real qkv work starts
            # at a ramped clock.
            for _ in range(10):
                wacc = pfill.tile([128, 512], F32, name="acc", tag="acc",
                                  bufs=1)
                nc.tensor.matmul(wacc, warm[:, 0:128], warm,
                                 start=True, stop=True)

            alt = [0]

            def drain_engine(allow_scalar=False):
                # GpSimd (Pool) cannot access PSUM.  ACT can take copies
                # only while exp traffic is light (early i-blocks);
                # otherwise it delays the attention critical path.
                alt[0] += 1
                if allow_scalar and alt[0] % 2 == 0:
                    return nc.scalar
                return nc.vector

            def do_copy(eng, dst, src):
                if eng is nc.scalar:
                    eng.copy(dst, src)
                else:
                    eng.tensor_copy(dst, src)

            def act_recip(out, in_):
                # Reciprocal on the ACT engine.  bass blocks it for general
                # use (reduced precision vs nc.vector.reciprocal), but for
                # softmax row-sums (~1e-3 rel) it is far inside this
                # kernel's bf16 error budget, and at the drain the ACT
                # queue is idle while the DVE recip would cost 2x3.2us.
                se = nc.scalar
                ins_ = [se.lower_ap(in_)]
                for val in (0.0, 1.0, 0.0):   # bias, scale, alpha
                    ins_.append(mybir.ImmediateValue(dtype=mybir.dt.float32,
                                                     value=val))
                se.add_instruction(mybir.InstActivation(
                    name=se.bass.get_next_instruction_name(),
                    func=mybir.ActivationFunctionType.Reciprocal,
                    ins=ins_, outs=[se.lower_ap(out)]))

            # ---- filler items: (kind, rows, emit_fn) ----
            def make_a_items(sb):
                """qkv projection work for s-block sb, as filler items."""
                xT = xTs[sb % 2]
                allow_sc = sb <= 1   # popped while exp traffic is light
                st = {}

                def dma_item():
                    if sb == 0:
                        st["xn"] = xn0s   # prefetched before the weights
                        return
                    xns = []
                    for st4 in range(4):
                        xn = pxn.tile([128, D], MMD, name="xn", tag="xn",
                                      bufs=4)
                        nc.gpsimd.dma_start(out=xn, in_=x_r[sb * 4 + st4])
                        xns.append(xn)
                    st["xn"] = xns

                def t_item(st4):
                    def emit():
                        xn = st["xn"][st4]
                        for g in range(2):
                            ptr4 = pfill.tile([128, 4, 128], MMD, name="ptr4",
                                              tag="ptr", bufs=1)
                            for k in range(4):
                                dt_ = 4 * g + k
                                nc.tensor.matmul(
                                    ptr4[:, k, :], xn[:, ts(dt_, 128)], const,
                                    is_transpose=True,
                                    start=(k == 0), stop=(k == 3),
                                )
                            do_copy(drain_engine(allow_sc),
                                    xT[:, 4 * g : 4 * g + 4, ts(st4, 128)],
                                    ptr4)
                    return emit

                def a_acc(direct):
                    # A(0)-direct runs BEFORE any attention: borrow the
                    # idle score banks as a double-buffered accumulator so
                    # back-to-back items don't serialize on drain WARs
                    # (which also reset the PE clock ramp)
                    if direct:
                        return psc.tile([128, 2, 512], F32, name="sc",
                                        tag="sc", bufs=2)[:, 0, :]
                    return pfill.tile([128, 512], F32, name="acc",
                                      tag="acc", bufs=1)

                def v_item(st4):
                    def emit():
                        acc = a_acc(sb == 0)
                        for dt_ in range(NDT):
                            nc.tensor.matmul(
                                acc, xT[:, dt_, ts(st4, 128)], wvt[:, dt_, :],
                                start=(dt_ == 0), stop=(dt_ == NDT - 1),
                            )
                        do_copy(drain_engine(allow_sc),
                                vA[:, sb * 4 + st4, :, 0:HD],
                                acc.rearrange("p (h c) -> p h c", h=HPC))
                    return emit

                def qk_item(w_t, dst_t, p):
                    def emit():
                        acc = a_acc(sb == 0 and p == 0)
                        for dt_ in range(NDT):
                            nc.tensor.matmul(
                                acc, w_t[:, dt_, ts(p, 128)], xT[:, dt_, :],
                                start=(dt_ == 0), stop=(dt_ == NDT - 1),
                            )
                        do_copy(drain_engine(allow_sc),
                                dst_t[:, p, ts(sb, 512)], acc)
                    return emit

                items = [("a", 0, dma_item)]
                # t/v interleaved: v(st4) needs only t(st4), so v work can
                # start while later x tiles are still streaming in
                for st4 in range(4):
                    items.append(("a", 1024, t_item(st4)))
                    items.append(("a", 4096, v_item(st4)))
                # k/q for head-pair p: needed by attention section (sb, p)
                for p in range(4):
                    items.append(("a", 4096, qk_item(wkt, kT, p)))
                    items.append(("a", 4096, qk_item(wqt, qT, p)))
                return items

            def make_p_items(ib, drain=False):
                """out-projection halves for i-block ib (after its norm).
                drain=True: borrow the (idle) score-pool banks as a
                double-buffered accumulator so back-to-back proj tiles
                don't serialize on the single pfill acc bank, and route
                copies/DMAs off the congested vector/sync queues."""
                def p_item(it, et):
                    def emit():
                        if drain:
                            acc = psc.tile([128, 2, 512], F32, name="sc",
                                           tag="sc", bufs=2)[:, 0, :]
                        else:
                            acc = pfill.tile([128, 512], F32, name="acc",
                                             tag="acc", bufs=1)
                        for ct in range(4):
                            nc.tensor.matmul(
                                acc,
                                outTb[ib][:, ct, ts(it - 4 * ib, 128)],
                                wpt[:, ct, et, :],
                                start=(ct == 0), stop=(ct == 3),
                            )
                        res = p3s.tile([128, 512], F32, name="res",
                                       tag="res", bufs=8)
                        if drain:
                            # ACT is idle after the last exp; keep the DVE
                            # free for the final norm muls.  Alternate the
                            # out DMAs across both queues so they drain in
                            # parallel at kernel exit
                            nc.scalar.copy(res, acc)
                            dq = nc.gpsimd if (it + et) % 2 else nc.sync
                            dq.dma_start(
                                out=out_r[it][:, ts(et, 512)], in_=res)
                        else:
                            drain_engine().tensor_copy(res, acc)
                            nc.sync.dma_start(
                                out=out_r[it][:, ts(et, 512)], in_=res)
                    return emit

                return [("p", 2048, p_item(it, et))
                        for it in range(4 * ib, 4 * ib + 4)
                        for et in range(2)]

            # ---- softmax normalization pipeline (batched per i-block) ----
            def norm_spread(p, oa01, nst):
                # the oc copy gates reuse of the single-buffered oa banks;
                # early i-blocks put it on the lightly-loaded ACT queue
                for half in range(2):
                    oc = p2n.tile([HD + 1, 512], F32, name="oc",
                                  tag="oc", bufs=10)
                    do_copy(nc.scalar if nst["ib"] <= 1 else nc.vector,
                            oc, oa01[half])
                    k = 2 * p + half
                    nc.sync.dma_start(
                        out=nst["spread"][:, 4 * k : 4 * k + 4],
                        in_=oc[HD : HD + 1, :])
                    nst["oc"][k] = oc

            def norm_recip(nst, dmae=None):
                dmae = dmae or nc.sync
                rcpt = p2n.tile([128, 32], F32, name="rcpt",
                                tag="rcpt", bufs=2)
                nc.vector.reciprocal(rcpt, nst["spread"])
                scr2 = p2d.tile([8 * 512], F32, name="scr2", tag="scr2")
                dmae.dma_start(
                    out=scr2.rearrange("(k q t) -> q k t", q=128, t=4),
                    in_=rcpt.rearrange("q (k t) -> q k t", t=4))
                rep_all = p2n.tile([HD, 8, 512], F32, name="rep_all",
                                   tag="rep_all", bufs=1)
                dmae.dma_start(
                    out=rep_all,
                    in_=scr2.rearrange("(k i) -> k i", k=8).unsqueeze(0)
                    .to_broadcast([HD, 8, 512]))
                nst["rep"] = rep_all

            def norm_muls_part(nst, ks):
                ib = nst["ib"]
                dst = outTb[ib]
                for k in ks:
                    p_, half = divmod(k, 2)
                    if half == 0:
                        nc.vector.tensor_mul(
                            dst[0:HD, p_, :],
                            nst["oc"][k][0:HD, :], nst["rep"][:, k, :])
                    else:
                        onsb = p2n.tile([HD, 512], MMD, name="onsb",
                                        tag="onsb", bufs=3)
                        nc.vector.tensor_mul(
                            onsb, nst["oc"][k][0:HD, :], nst["rep"][:, k, :])
                        nc.sync.dma_start(
                            out=dst[HD : 2 * HD, p_, :], in_=onsb)

            # ---- the fused stream ----
            filler = deque()
            a_left = [0]        # unpopped qkv-projection (A) items
            rows_done = [0]     # PE rows emitted via fillers
            glb = [1]           # global j-tile counter (pacing clock)
            ROWS_PER_JT = 1550  # filler rows/j-tile to cover the exp deficit

            def pop_one():
                kind, rows, fn = filler.popleft()
                if kind == "a":
                    a_left[0] -= 1
                rows_done[0] += rows
                fn()

            # A(0): x DMAs + transposes + v directly (attention i-block 0
            # needs them immediately); k/q head-pairs 1..3 become fillers
            # popped during the first attention sections.
            a0 = make_a_items(0)
            for kind, rows, fn in a0[:11]:   # dma, 4xT, 4xV, k0, q0
                fn()
            filler.extend(a0[11:])
            a_left[0] += len(a0) - 11

            nq = {}
            for ib in range(NSB):
                if ib + 1 < NSB:
                    items = make_a_items(ib + 1)
                    a_left[0] += len(items)
                    filler.extend(items)
                njt = 4 * (ib + 1)
                blk_jts = 4 * njt
                jts_left_blk = [blk_jts]
                a_blk = a_left[0]
                for p in range(4):
                    if p == 0:
                        nq[ib] = {"ib": ib, "oc": [None] * 8,
                                  "spread": p2n.tile([128, 32], F32,
                                                     name="spread",
                                                     tag="spread", bufs=2)}
                        if ib >= 1:
                            norm_recip(nq[ib - 1])
                    oa01 = [poa.tile([HD + 1, 512], F32, name=f"oa{h}",
                                     tag=f"oa{h}", bufs=1) for h in range(2)]
                    avq = deque()
                    for jt in range(njt):
                        d = jt - 4 * ib  # diagonal index; <0 => full block
                        off = 0 if d < 0 else 128 * d
                        sc = psc.tile([128, 2, 512], F32, name="sc",
                                      tag="sc", bufs=2)
                        for half in range(2):
                            hsl = slice(half * HD, half * HD + HD)
                            nc.tensor.matmul(
                                sc[:, half, off:],
                                kT[hsl, p, ts(jt, 128)],
                                qT[hsl, p, ib * 512 + off : (ib + 1) * 512],
                                start=True, stop=True,
                            )
                        at2 = pat.tile([128, 2, 512], MMD, name="at2",
                                       tag="at2", bufs=4)
                        nc.scalar.activation(
                            at2[:, :, off:], sc[:, :, off:], EXP)
                        if d >= 0:
                            # causal mask on the diagonal 128-col block only
                            # (cols beyond it are fully below the diagonal)
                            nc.gpsimd.affine_select(
                                out=at2[:, :, off : off + 128],
                                in_=at2[:, :, off : off + 128],
                                compare_op=mybir.AluOpType.is_ge,
                                fill=0.0, base=ib * 512 + off - jt * 128,
                                pattern=[[0, 2], [1, 128]],
                                channel_multiplier=-1,
                            )
                        # software pipeline: attn@v lags TWO j-tiles behind
                        # the scores so momentary exp/select lag on the
                        # ACT/Pool queues never stalls the PE
                        if len(avq) >= 2:
                            pjt, pat2, poff = avq.popleft()
                            for half in range(2):
                                nc.tensor.matmul(
                                    oa01[half][:, poff:],
                                    vA[:, pjt, 2 * p + half, :],
                                    pat2[:, half, poff:],
                                    start=(pjt == 0), stop=False,
                                )
                        avq.append((jt, at2, off))
                        # pacing: fillers between j-tiles keep the PE busy
                        # through the exp latency; all pending A items must
                        # fully pop before this i-block ends (the next one
                        # consumes their outputs)
                        # cap pops per j-tile: back-to-back filler items
                        # serialize on the single acc bank's drain copy
                        jts_left_blk[0] -= 1
                        target_a = a_blk * jts_left_blk[0] // blk_jts
                        popped = 0
                        while (filler and a_left[0] > target_a
                               and popped < 2):
                            pop_one()
                            popped += 1
                        if (filler and popped == 0
                                and rows_done[0] < ROWS_PER_JT * glb[0]):
                            pop_one()
                        glb[0] += 1
                    # flush pending attn@v (closes both accumulations)
                    while avq:
                        pjt, pat2, poff = avq.popleft()
                        for half in range(2):
                            nc.tensor.matmul(
                                oa01[half][:, poff:],
                                vA[:, pjt, 2 * p + half, :],
                                pat2[:, half, poff:],
                                start=(pjt == 0), stop=(not avq),
                            )
                    norm_spread(p, oa01, nq[ib])
                    if ib == 3 and p < 3:
                        # last i-block: per-SECTION norm so only head-pair
                        # 3's short chain remains exposed at the drain.
                        # Writes go to outTb[3].
                        rcp8 = p2n.tile([128, 8], F32, name="rcp8",
                                        tag="rcp8", bufs=2)
                        nc.vector.reciprocal(
                            rcp8, nq[3]["spread"][:, 8 * p : 8 * p + 8])
                        scr2p = p2d.tile([2 * 512], F32, name="scr2p",
                                         tag="scr2p")
                        nc.gpsimd.dma_start(
                            out=scr2p.rearrange("(k q t) -> q k t",
                                                q=128, t=4),
                            in_=rcp8.rearrange("q (k t) -> q k t", t=4))
                        repp = p2n.tile([HD, 2, 512], F32, name="repp",
                                        tag="repp", bufs=2)
                        nc.gpsimd.dma_start(
                            out=repp,
                            in_=scr2p.rearrange("(k i) -> k i", k=2)
                            .unsqueeze(0).to_broadcast([HD, 2, 512]))
                        for half in range(2):
                            k = 2 * p + half
                            if half == 0:
                                nc.vector.tensor_mul(
                                    outTb[3][0:HD, p, :],
                                    nq[3]["oc"][k][0:HD, :], repp[:, half, :])
                            else:
                                onsb = p2n.tile([HD, 512], MMD, name="onsb",
                                                tag="onsb", bufs=3)
                                nc.vector.tensor_mul(
                                    onsb, nq[3]["oc"][k][0:HD, :],
                                    repp[:, half, :])
                                nc.gpsimd.dma_start(
                                    out=outTb[3][HD : 2 * HD, p, :], in_=onsb)
                    elif ib == 3:
                        # final head-pair, fully exposed at the drain: no
                        # DMA bounce.  Exact reciprocal straight on the
                        # [1,512] sums row, then a tiny PE matmul
                        # (ones64^T @ recip_row) broadcasts it across the
                        # 64 hd partitions into the now-idle score banks.
                        # (reciprocal_approx_fast returns garbage in this
                        # runtime -- custom-DVE table not loaded.)
                        for half in range(2):
                            k = 2 * p + half
                            oc_k = nq[3]["oc"][k]
                            rr = p2n.tile([1, 512], F32, name="rr",
                                          tag="rr", bufs=2)
                            act_recip(rr, oc_k[HD : HD + 1, :])
                            repp3 = psc.tile([128, 2, 512], F32, name="sc",
                                             tag="sc", bufs=2)[0:HD, 0, :]
                            nc.tensor.matmul(repp3, ones64, rr,
                                             start=True, stop=True)
                            if half == 0:
                                nc.vector.tensor_mul(
                                    outTb[3][0:HD, p, :], oc_k[0:HD, :], repp3)
                            else:
                                onsb = p2n.tile([HD, 512], MMD, name="onsb",
                                                tag="onsb", bufs=3)
                                nc.vector.tensor_mul(
                                    onsb, oc_k[0:HD, :], repp3)
                                nc.gpsimd.dma_start(
                                    out=outTb[3][HD : 2 * HD, p, :], in_=onsb)
                    # spread the previous block's 8 norm muls across the
                    # next three section boundaries: dumping all of them
                    # onto the DVE at once queues them ahead of the filler
                    # drains the PE is about to need (~4us stalls)
                    if p == 0 and ib >= 1:
                        norm_muls_part(nq[ib - 1], [0, 1, 2])
                    if p == 1 and ib >= 1:
                        norm_muls_part(nq[ib - 1], [3, 4, 5])
                    if p == 2 and ib >= 1:
                        norm_muls_part(nq[ib - 1], [6, 7])
                        if ib == 3:
                            # hold two P(2) tiles back as PE fill for the
                            # final head-pair's reciprocal chain
                            filler.extend(make_p_items(2)[:6])
                            held = make_p_items(2, drain=True)[6:]
                        else:
                            filler.extend(make_p_items(ib - 1))
            # drain: only the final out-projection remains; all of the last
            # i-block's normalization already ran per-section
            while filler:
                pop_one()
            for _, _, fn in held:
                fn()
            for _, _, fn in make_p_items(3, drain=True):
                fn()


def build():
    nc = bacc.Bacc("TRN2", target_bir_lowering=False, debug=False,
                   num_devices=NCORES)
    io = {
        "x": nc.dram_tensor("x", [S, D], MMD, kind="ExternalInput").ap(),
        "wq": nc.dram_tensor("wq", [D, FG], MMD, kind="ExternalInput").ap(),
        "wk": nc.dram_tensor("wk", [D, FG], MMD, kind="ExternalInput").ap(),
        "wv": nc.dram_tensor("wv", [D, FG], MMD, kind="ExternalInput").ap(),
        "wp": nc.dram_tensor("wp", [FG, D], MMD, kind="ExternalInput").ap(),
        "ident": nc.dram_tensor("ident", [128, 128], MMD,
                                kind="ExternalInput").ap(),
        "out": nc.dram_tensor("out", [S, D], F32, kind="ExternalOutput").ap(),
    }
    with tile.TileContext(nc) as tc:
        _body(tc, io)
    nc.compile()
    return nc


def _host_inputs(x, W_attn, b_attn, W_proj):
    import ml_dtypes

    bf16 = ml_dtypes.bfloat16
    assert not np.any(b_attn), "kernel assumes b_attn == 0 (spec fill: zeros)"
    ident = np.eye(128, dtype=bf16)
    in_maps = []
    for c in range(NCORES):
        b, g = divmod(c, 2)
        in_maps.append({
            "x": np.asarray(x[b], dtype=bf16),
            # fold the 1/sqrt(HD) score scale into wq (exact: * 2^-3)
            "wq": np.asarray(
                W_attn[:, g * FG : (g + 1) * FG] * np.float32(0.125),
                dtype=bf16),
            "wk": np.asarray(
                W_attn[:, D + g * FG : D + (g + 1) * FG], dtype=bf16),
            "wv": np.asarray(
                W_attn[:, 2 * D + g * FG : 2 * D + (g + 1) * FG], dtype=bf16),
            "wp": np.asarray(W_proj[g * FG : (g + 1) * FG, :], dtype=bf16),
            "ident": ident,
        })
    return in_maps


_NC_CACHE = {}


def kernel(x, W_attn, b_attn, W_proj, b_proj, _trace=False):
    x = np.asarray(x)
    W_attn = np.asarray(W_attn)
    b_attn = np.asarray(b_attn)
    W_proj = np.asarray(W_proj)
    b_proj = np.asarray(b_proj)

    if "nc" not in _NC_CACHE:
        _NC_CACHE["nc"] = build()
    nc = _NC_CACHE["nc"]

    in_maps = _host_inputs(x, W_attn, b_attn, W_proj)
    kwargs = {}
    if _trace:
        _install_ntff_hook()
        kwargs = dict(trace=True, trace_cores=[0])
    res = run_bass_kernel_spmd(nc, in_maps, core_ids=list(range(NCORES)),
                               **kwargs)
    y = np.empty((B, S, D), dtype=np.float32)
    for b in range(B):
        y[b] = (res.results[2 * b]["out"] + res.results[2 * b + 1]["out"]
                + b_proj.astype(np.float32))
    if _trace:
        kernel.last_exec_time_ns = res.exec_time_ns
        kernel.last_trace = res.instructions_and_trace
    return y


# revision 74
# speedup vs baseline: 1.0291x; 1.0291x over previous
"""Causal multi-head attention block on 8 Trainium2 NeuronCores.

Problem: x[4,2048,1024] -> qkv proj -> 16-head causal attention -> out proj.

Sharding: 8 cores = 4 batches x 2 head-groups (8 heads each). Each core
computes, for its (batch, head-group):
  - qT/kT (feature-on-partition, via PE-transposed x) and v (natural layout)
  - causal attention with scores computed transposed (scoresT[j, i]):
    softmax without max-subtraction (scores are O(1) for these inputs),
    row-sums from an appended ones-column on v in the attn@v matmul
  - partial out-projection with its 512 rows of W_proj
Host sums the two partials per batch and adds b_proj.

Perf structure — one fused PE stream (v3):
  - all matmul operands bfloat16 (1 c/r at any N, ~half the PE power of
    f32r so the GPIO power throttle stays quiet); PSUM accumulation f32
  - the attention phase is ACT-bound per j-tile (exp ~1.28us vs ~0.85us
    of PE work), so the qkv projection and out-projection are NOT separate
    phases: they are a queue of small "filler" bursts (transpose group /
    v-tile / qk-tile / proj-half) popped between attention j-tiles.  The
    PE therefore never stalls on exp, which also keeps the PE DVFS ramp
    at its 2.4 GHz pstate (isolated matmul bursts run at 1.2 GHz).
  - qkv for s-block sb is produced as filler during attention i-block
    sb-1; out-proj for i-block ib runs as filler after its softmax
    normalization completes (during i-block ib+1)
  - scores sc pool double-buffered; emission order sc(jt) -> exp(jt) ->
    mask(jt) -> av(jt-1) software-pipelines the exp latency
  - exp on ACT only; causal mask via GpSimd affine_select on the
    diagonal 128-col block; diagonal blocks fully N-trimmed (bf16)
  - softmax reciprocal on a [64,8]-spread layout (DRAM bounce) instead of
    a [1,512] single-partition op (3.3us -> ~50ns on DVE); the LAST
    i-block normalizes per-section (hidden under attention), and its
    final head-pair skips the DMA bounce entirely: reciprocal on the
    (drain-idle) ACT engine + a PE partition-broadcast matmul
  - PSUM budget exactly 8 banks: sc 2x2 + oa 2x1 + transpose 1 + shared
    qkv/proj accumulator 1
  - normalized output in per-i-block tiles (coarse dep tracking makes a
    shared tile's norm writes false barriers for filler reads); the 8
    norm muls spread over three section boundaries so the DVE queue never
    blocks the filler drains the PE needs next
Measured end-to-end rel err vs the fp64 reference: ~3.4e-3 (bf16).
HW exec ~294us (thermally noisy up to ~10%; f32r two-phase baseline was
~390-406us).
"""

import sys
import types as _types
from collections import deque

import numpy as np

import concourse.mybir as mybir
import concourse.tile as tile
from concourse import bacc
from concourse.bass import ts
from concourse.bass_utils import run_bass_kernel_spmd

# ---- problem constants (hardcoded per harness contract) ----
B, S, D, H = 4, 2048, 1024, 16
HD = D // H            # 64 head dim
HPC = H // 2           # 8 heads per core
FG = HPC * HD          # 512 features per head-group
NCORES = 8
NST = S // 128         # 16 s-tiles
NDT = D // 128         # 8 d-tiles
NSB = S // 512         # 4 s/i-blocks

F32 = mybir.dt.float32
MMD = mybir.dt.bfloat16
EXP = mybir.ActivationFunctionType.Exp


def _install_ntff_hook():
    """run_bass_kernel_spmd(trace=True) under axon needs antenv.axon_hooks,
    absent in this image; shim it with the boot module's ctypes hook."""
    if "antenv.axon_hooks" in sys.modules:
        return
    try:
        from trn_agent_boot.trn_boot import _ntff_profile_via_ctypes
    except ImportError:
        return
    m = _types.ModuleType("antenv.axon_hooks")
    m.get_axon_ntff_profile_hook = lambda: _ntff_profile_via_ctypes(
        "/opt/axon/libaxon_pjrt.so"
    )
    m.set_axon_ntff_profile_hook = lambda h: None
    sys.modules["antenv.axon_hooks"] = m


def _body(tc, io):
    nc = tc.nc
    x_r = io["x"].rearrange("(st p) d -> st p d", p=128)      # [16,128,1024]
    wq_r = io["wq"].rearrange("(dt p) f -> dt p f", p=128)    # [8,128,512]
    wk_r = io["wk"].rearrange("(dt p) f -> dt p f", p=128)
    wv_r = io["wv"].rearrange("(dt p) f -> dt p f", p=128)
    wp_r = io["wp"].rearrange("(ct p) e -> ct p e", p=128)    # [4,128,1024]
    out_r = io["out"].rearrange("(st p) e -> st p e", p=128)  # [16,128,1024]

    with tc.tile_pool(name="persist", bufs=1) as pp:
        qT = pp.tile([128, 4, S], MMD, name="qT")             # [f, pair, s]
        kT = pp.tile([128, 4, S], MMD, name="kT")
        vA = pp.tile([128, NST, HPC, HD + 1], MMD, name="vA")  # v | ones
        # normalized attention output, one tile PER i-block: dependency
        # tracking on a shared tile is coarse, so any norm write would act
        # as a false barrier for later-emitted out-proj reads of OTHER
        # blocks running as fillers
        outTb = [pp.tile([128, 4, 512], MMD, name=f"outTb{i}")
                 for i in range(NSB)]
        const = pp.tile([128, 128], MMD, name="const")        # identity
        wvt = pp.tile([128, NDT, 512], MMD, name="wvt")
        wqt = pp.tile([128, NDT, 512], MMD, name="wqt")
        wkt = pp.tile([128, NDT, 512], MMD, name="wkt")
        wpt = pp.tile([128, 4, 2, 512], MMD, name="wpt")
        # x^T ping-pong: A(sb) writes xTs[sb%2] while A(sb-1)'s last
        # consumers may still be in flight
        xTs = [pp.tile([128, NDT, 512], MMD, name=f"xT{i}") for i in range(2)]

        nc.sync.dma_start(out=const, in_=io["ident"])

        # broadcast-copy a 1.0 constant into the ones column
        ones1 = pp.tile([128, 1], F32, name="ones1")
        nc.vector.memset(ones1, 1.0)
        nc.vector.tensor_copy(
            vA[:, :, :, HD : HD + 1],
            ones1.unsqueeze(1).to_broadcast([128, NST, HPC, 1]),
        )
        # [1,64] ones row: stationary for the drain-time PE partition-
        # broadcast of the last head-pair's softmax reciprocals
        ones64 = pp.tile([1, HD], F32, name="ones64")
        nc.vector.tensor_copy(
            ones64, ones1[0:1, 0:1].to_broadcast([1, HD]))
        # scratch operand for the PE clock pre-ramp (see below)
        warm = pp.tile([128, 512], MMD, name="warm")
        nc.vector.memset(warm, 1.0)

        with (
            tc.tile_pool(name="pxn", bufs=4) as pxn,
            tc.tile_pool(name="pfill", bufs=1, space="PSUM") as pfill,
            tc.tile_pool(name="psc", bufs=2, space="PSUM") as psc,
            tc.tile_pool(name="poa", bufs=1, space="PSUM") as poa,
            tc.tile_pool(name="pat", bufs=4) as pat,
            tc.tile_pool(name="p2n", bufs=2) as p2n,
            tc.tile_pool(name="p2_dr", bufs=4, space="DRAM") as p2d,
            tc.tile_pool(name="p3_r", bufs=4) as p3s,
        ):
            # x(sb=0) prefetch FIRST on gpsimd — the first transposes need
            # it; v/q weights stream on sync in parallel, k weights behind
            # the prefetch on gpsimd
            xn0s = []
            for st4 in range(4):
                xn = pxn.tile([128, D], MMD, name="xn", tag="xn", bufs=4)
                nc.gpsimd.dma_start(out=xn, in_=x_r[st4])
                xn0s.append(xn)
            for dt_ in range(NDT):
                nc.sync.dma_start(out=wvt[:, dt_, :], in_=wv_r[dt_])
                nc.gpsimd.dma_start(out=wkt[:, dt_, :], in_=wk_r[dt_])
            for dt_ in range(NDT):
                nc.sync.dma_start(out=wqt[:, dt_, :], in_=wq_r[dt_])
            for ct in range(4):
                for et in range(2):
                    nc.sync.dma_start(out=wpt[:, ct, et, :],
                                      in_=wp_r[ct][:, ts(et, 512)])

            # PE clock pre-ramp: the DVFS pstate needs ~3us of continuous
            # activity to leave the 0.65/1.2 GHz states; the PE is
            # otherwise idle until the first x tile lands (~11us).  Dummy
            # matmuls on the memset scratch (results never read) keep the
            # array busy through that window so the real qkv work starts
            # at a ramped clock.
            for _ in range(10):
                wacc = pfill.tile([128, 512], F32, name="acc", tag="acc",
                                  bufs=1)
                nc.tensor.matmul(wacc, warm[:, 0:128], warm,
                                 start=True, stop=True)

            alt = [0]

            def drain_engine(allow_scalar=False):
                # GpSimd (Pool) cannot access PSUM.  ACT can take copies
                # only while exp traffic is light (early i-blocks);
                # otherwise it delays the attention critical path.
                alt[0] += 1
                if allow_scalar and alt[0] % 2 == 0:
                    return nc.scalar
                return nc.vector

            def do_copy(eng, dst, src):
                if eng is nc.scalar:
                    eng.copy(dst, src)
                else:
                    eng.tensor_copy(dst, src)

            def act_recip(out, in_):
                # Reciprocal on the ACT engine.  bass blocks it for general
                # use (reduced precision vs nc.vector.reciprocal), but for
                # softmax row-sums (~1e-3 rel) it is far inside this
                # kernel's bf16 error budget, and at the drain the ACT
                # queue is idle while the DVE recip would cost 2x3.2us.
                se = nc.scalar
                ins_ = [se.lower_ap(in_)]
                for val in (0.0, 1.0, 0.0):   # bias, scale, alpha
                    ins_.append(mybir.ImmediateValue(dtype=mybir.dt.float32,
                                                     value=val))
                se.add_instruction(mybir.InstActivation(
                    name=se.bass.get_next_instruction_name(),
                    func=mybir.ActivationFunctionType.Reciprocal,
                    ins=ins_, outs=[se.lower_ap(out)]))

            # ---- filler items: (kind, rows, emit_fn) ----
            def make_a_items(sb):
                """qkv projection work for s-block sb, as filler items."""
                xT = xTs[sb % 2]
                allow_sc = sb <= 1   # popped while exp traffic is light
                st = {}

                def dma_item():
                    if sb == 0:
                        st["xn"] = xn0s   # prefetched before the weights
                        return
                    xns = []
                    for st4 in range(4):
                        xn = pxn.tile([128, D], MMD, name="xn", tag="xn",
                                      bufs=4)
                        nc.gpsimd.dma_start(out=xn, in_=x_r[sb * 4 + st4])
                        xns.append(xn)
                    st["xn"] = xns

                def t_item(st4):
                    def emit():
                        xn = st["xn"][st4]
                        for g in range(2):
                            ptr4 = pfill.tile([128, 4, 128], MMD, name="ptr4",
                                              tag="ptr", bufs=1)
                            for k in range(4):
                                dt_ = 4 * g + k
                                nc.tensor.matmul(
                                    ptr4[:, k, :], xn[:, ts(dt_, 128)], const,
                                    is_transpose=True,
                                    start=(k == 0), stop=(k == 3),
                                )
                            do_copy(drain_engine(allow_sc),
                                    xT[:, 4 * g : 4 * g + 4, ts(st4, 128)],
                                    ptr4)
                    return emit

                def a_acc(direct):
                    # A(0)-direct runs BEFORE any attention: borrow the
                    # idle score banks as a double-buffered accumulator so
                    # back-to-back items don't serialize on drain WARs
                    # (which also reset the PE clock ramp)
                    if direct:
                        return psc.tile([128, 2, 512], F32, name="sc",
                                        tag="sc", bufs=2)[:, 0, :]
                    return pfill.tile([128, 512], F32, name="acc",
                                      tag="acc", bufs=1)

                def v_item(st4):
                    def emit():
                        acc = a_acc(sb == 0)
                        for dt_ in range(NDT):
                            nc.tensor.matmul(
                                acc, xT[:, dt_, ts(st4, 128)], wvt[:, dt_, :],
                                start=(dt_ == 0), stop=(dt_ == NDT - 1),
                            )
                        do_copy(drain_engine(allow_sc),
                                vA[:, sb * 4 + st4, :, 0:HD],
                                acc.rearrange("p (h c) -> p h c", h=HPC))
                    return emit

                def qk_item(w_t, dst_t, p):
                    def emit():
                        acc = a_acc(sb == 0 and p == 0)
                        for dt_ in range(NDT):
                            nc.tensor.matmul(
                                acc, w_t[:, dt_, ts(p, 128)], xT[:, dt_, :],
                                start=(dt_ == 0), stop=(dt_ == NDT - 1),
                            )
                        do_copy(drain_engine(allow_sc),
                                dst_t[:, p, ts(sb, 512)], acc)
                    return emit

                items = [("a", 0, dma_item)]
                # t/v interleaved: v(st4) needs only t(st4), so v work can
                # start while later x tiles are still streaming in
                for st4 in range(4):
                    items.append(("a", 1024, t_item(st4)))
                    items.append(("a", 4096, v_item(st4)))
                # k/q for head-pair p: needed by attention section (sb, p)
                for p in range(4):
                    items.append(("a", 4096, qk_item(wkt, kT, p)))
                    items.append(("a", 4096, qk_item(wqt, qT, p)))
                return items

            def make_p_items(ib, drain=False):
                """out-projection halves for i-block ib (after its norm).
                drain=True: borrow the (idle) score-pool banks as a
                double-buffered accumulator so back-to-back proj tiles
                don't serialize on the single pfill acc bank, and route
                copies/DMAs off the congested vector/sync queues."""
                def p_item(it, et):
                    def emit():
                        if drain:
                            acc = psc.tile([128, 2, 512], F32, name="sc",
                                           tag="sc", bufs=2)[:, 0, :]
                        else:
                            acc = pfill.tile([128, 512], F32, name="acc",
                                             tag="acc", bufs=1)
                        for ct in range(4):
                            nc.tensor.matmul(
                                acc,
                                outTb[ib][:, ct, ts(it - 4 * ib, 128)],
                                wpt[:, ct, et, :],
                                start=(ct == 0), stop=(ct == 3),
                            )
                        res = p3s.tile([128, 512], F32, name="res",
                                       tag="res", bufs=8)
                        if drain:
                            # ACT is idle after the last exp; keep the DVE
                            # free for the final norm muls.  Alternate the
                            # out DMAs across both queues so they drain in
                            # parallel at kernel exit
                            nc.scalar.copy(res, acc)
                            dq = nc.gpsimd if (it + et) % 2 else nc.sync
                            dq.dma_start(
                                out=out_r[it][:, ts(et, 512)], in_=res)
                        else:
                            drain_engine().tensor_copy(res, acc)
                            nc.sync.dma_start(
                                out=out_r[it][:, ts(et, 512)], in_=res)
                    return emit

                return [("p", 2048, p_item(it, et))
                        for it in range(4 * ib, 4 * ib + 4)
                        for et in range(2)]

            # ---- softmax normalization pipeline (batched per i-block) ----
            def norm_spread(p, oa01, nst):
                # the oc copy gates reuse of the single-buffered oa banks;
                # early i-blocks put it on the lightly-loaded ACT queue
                for half in range(2):
                    oc = p2n.tile([HD + 1, 512], F32, name="oc",
                                  tag="oc", bufs=10)
                    do_copy(nc.scalar if nst["ib"] <= 1 else nc.vector,
                            oc, oa01[half])
                    k = 2 * p + half
                    nc.sync.dma_start(
                        out=nst["spread"][:, 4 * k : 4 * k + 4],
                        in_=oc[HD : HD + 1, :])
                    nst["oc"][k] = oc

            def norm_recip(nst, dmae=None):
                dmae = dmae or nc.sync
                rcpt = p2n.tile([128, 32], F32, name="rcpt",
                                tag="rcpt", bufs=2)
                nc.vector.reciprocal(rcpt, nst["spread"])
                scr2 = p2d.tile([8 * 512], F32, name="scr2", tag="scr2")
                dmae.dma_start(
                    out=scr2.rearrange("(k q t) -> q k t", q=128, t=4),
                    in_=rcpt.rearrange("q (k t) -> q k t", t=4))
                rep_all = p2n.tile([HD, 8, 512], F32, name="rep_all",
                                   tag="rep_all", bufs=1)
                dmae.dma_start(
                    out=rep_all,
                    in_=scr2.rearrange("(k i) -> k i", k=8).unsqueeze(0)
                    .to_broadcast([HD, 8, 512]))
                nst["rep"] = rep_all

            def norm_mul_items(nst, ks):
                items = []
                ib = nst["ib"]
                dst = outTb[ib]
                def one(k):
                    def emit():
                        p_, half = divmod(k, 2)
                        if half == 0:
                            nc.vector.tensor_mul(
                                dst[0:HD, p_, :],
                                nst["oc"][k][0:HD, :], nst["rep"][:, k, :])
                        else:
                            onsb = p2n.tile([HD, 512], MMD, name="onsb",
                                            tag="onsb", bufs=3)
                            nc.vector.tensor_mul(
                                onsb, nst["oc"][k][0:HD, :],
                                nst["rep"][:, k, :])
                            nc.sync.dma_start(
                                out=dst[HD : 2 * HD, p_, :], in_=onsb)
                    return emit
                for k in ks:
                    items.append(one(k))
                return items

            # ---- the fused stream ----
            filler = deque()
            mulq = deque()  # pending norm muls, drained one per j-tile
            a_left = [0]        # unpopped qkv-projection (A) items
            rows_done = [0]     # PE rows emitted via fillers
            glb = [1]           # global j-tile counter (pacing clock)
            ROWS_PER_JT = 1550  # filler rows/j-tile to cover the exp deficit

            def pop_one():
                kind, rows, fn = filler.popleft()
                if kind == "a":
                    a_left[0] -= 1
                rows_done[0] += rows
                fn()

            # A(0): x DMAs + transposes + v directly (attention i-block 0
            # needs them immediately); k/q head-pairs 1..3 become fillers
            # popped during the first attention sections.
            a0 = make_a_items(0)
            for kind, rows, fn in a0[:11]:   # dma, 4xT, 4xV, k0, q0
                fn()
            filler.extend(a0[11:])
            a_left[0] += len(a0) - 11

            nq = {}
            for ib in range(NSB):
                if ib + 1 < NSB:
                    items = make_a_items(ib + 1)
                    a_left[0] += len(items)
                    filler.extend(items)
                njt = 4 * (ib + 1)
                blk_jts = 4 * njt
                jts_left_blk = [blk_jts]
                a_blk = a_left[0]
                for p in range(4):
                    if p == 0:
                        nq[ib] = {"ib": ib, "oc": [None] * 8,
                                  "spread": p2n.tile([128, 32], F32,
                                                     name="spread",
                                                     tag="spread", bufs=2)}
                        if ib >= 1:
                            norm_recip(nq[ib - 1])
                    oa01 = [poa.tile([HD + 1, 512], F32, name=f"oa{h}",
                                     tag=f"oa{h}", bufs=1) for h in range(2)]
                    avq = deque()
                    for jt in range(njt):
                        d = jt - 4 * ib  # diagonal index; <0 => full block
                        off = 0 if d < 0 else 128 * d
                        sc = psc.tile([128, 2, 512], F32, name="sc",
                                      tag="sc", bufs=2)
                        for half in range(2):
                            hsl = slice(half * HD, half * HD + HD)
                            nc.tensor.matmul(
                                sc[:, half, off:],
                                kT[hsl, p, ts(jt, 128)],
                                qT[hsl, p, ib * 512 + off : (ib + 1) * 512],
                                start=True, stop=True,
                            )
                        at2 = pat.tile([128, 2, 512], MMD, name="at2",
                                       tag="at2", bufs=4)
                        nc.scalar.activation(
                            at2[:, :, off:], sc[:, :, off:], EXP)
                        if d >= 0:
                            # causal mask on the diagonal 128-col block only
                            # (cols beyond it are fully below the diagonal)
                            nc.gpsimd.affine_select(
                                out=at2[:, :, off : off + 128],
                                in_=at2[:, :, off : off + 128],
                                compare_op=mybir.AluOpType.is_ge,
                                fill=0.0, base=ib * 512 + off - jt * 128,
                                pattern=[[0, 2], [1, 128]],
                                channel_multiplier=-1,
                            )
                        # software pipeline: attn@v lags TWO j-tiles behind
                        # the scores so momentary exp/select lag on the
                        # ACT/Pool queues never stalls the PE
                        if len(avq) >= 2:
                            pjt, pat2, poff = avq.popleft()
                            for half in range(2):
                                nc.tensor.matmul(
                                    oa01[half][:, poff:],
                                    vA[:, pjt, 2 * p + half, :],
                                    pat2[:, half, poff:],
                                    start=(pjt == 0), stop=False,
                                )
                        avq.append((jt, at2, off))
                        # pacing: fillers between j-tiles keep the PE busy
                        # through the exp latency; all pending A items must
                        # fully pop before this i-block ends (the next one
                        # consumes their outputs)
                        # cap pops per j-tile: back-to-back filler items
                        # serialize on the single acc bank's drain copy
                        jts_left_blk[0] -= 1
                        target_a = a_blk * jts_left_blk[0] // blk_jts
                        popped = 0
                        while (filler and a_left[0] > target_a
                               and popped < 2):
                            pop_one()
                            popped += 1
                        if (filler and popped == 0
                                and rows_done[0] < ROWS_PER_JT * glb[0]):
                            pop_one()
                        if mulq:
                            mulq.popleft()()
                        glb[0] += 1
                    # flush pending attn@v (closes both accumulations)
                    while avq:
                        pjt, pat2, poff = avq.popleft()
                        for half in range(2):
                            nc.tensor.matmul(
                                oa01[half][:, poff:],
                                vA[:, pjt, 2 * p + half, :],
                                pat2[:, half, poff:],
                                start=(pjt == 0), stop=(not avq),
                            )
                    norm_spread(p, oa01, nq[ib])
                    if ib == 3 and p < 3:
                        # last i-block: per-SECTION norm so only head-pair
                        # 3's short chain remains exposed at the drain.
                        # Writes go to outTb[3].
                        rcp8 = p2n.tile([128, 8], F32, name="rcp8",
                                        tag="rcp8", bufs=2)
                        nc.vector.reciprocal(
                            rcp8, nq[3]["spread"][:, 8 * p : 8 * p + 8])
                        scr2p = p2d.tile([2 * 512], F32, name="scr2p",
                                         tag="scr2p")
                        nc.gpsimd.dma_start(
                            out=scr2p.rearrange("(k q t) -> q k t",
                                                q=128, t=4),
                            in_=rcp8.rearrange("q (k t) -> q k t", t=4))
                        repp = p2n.tile([HD, 2, 512], F32, name="repp",
                                        tag="repp", bufs=2)
                        nc.gpsimd.dma_start(
                            out=repp,
                            in_=scr2p.rearrange("(k i) -> k i", k=2)
                            .unsqueeze(0).to_broadcast([HD, 2, 512]))
                        for half in range(2):
                            k = 2 * p + half
                            if half == 0:
                                nc.vector.tensor_mul(
                                    outTb[3][0:HD, p, :],
                                    nq[3]["oc"][k][0:HD, :], repp[:, half, :])
                            else:
                                onsb = p2n.tile([HD, 512], MMD, name="onsb",
                                                tag="onsb", bufs=3)
                                nc.vector.tensor_mul(
                                    onsb, nq[3]["oc"][k][0:HD, :],
                                    repp[:, half, :])
                                nc.gpsimd.dma_start(
                                    out=outTb[3][HD : 2 * HD, p, :], in_=onsb)
                    elif ib == 3:
                        # final head-pair, fully exposed at the drain: no
                        # DMA bounce.  Exact reciprocal straight on the
                        # [1,512] sums row, then a tiny PE matmul
                        # (ones64^T @ recip_row) broadcasts it across the
                        # 64 hd partitions into the now-idle score banks.
                        # (reciprocal_approx_fast returns garbage in this
                        # runtime -- custom-DVE table not loaded.)
                        for half in range(2):
                            k = 2 * p + half
                            oc_k = nq[3]["oc"][k]
                            rr = p2n.tile([1, 512], F32, name="rr",
                                          tag="rr", bufs=2)
                            act_recip(rr, oc_k[HD : HD + 1, :])
                            repp3 = psc.tile([128, 2, 512], F32, name="sc",
                                             tag="sc", bufs=2)[0:HD, 0, :]
                            nc.tensor.matmul(repp3, ones64, rr,
                                             start=True, stop=True)
                            if half == 0:
                                nc.vector.tensor_mul(
                                    outTb[3][0:HD, p, :], oc_k[0:HD, :], repp3)
                            else:
                                onsb = p2n.tile([HD, 512], MMD, name="onsb",
                                                tag="onsb", bufs=3)
                                nc.vector.tensor_mul(
                                    onsb, oc_k[0:HD, :], repp3)
                                nc.gpsimd.dma_start(
                                    out=outTb[3][HD : 2 * HD, p, :], in_=onsb)
                    # spread the previous block's 8 norm muls across the
                    # next three section boundaries: dumping all of them
                    # onto the DVE at once queues them ahead of the filler
                    # drains the PE is about to need (~4us stalls)
                    if p == 0 and ib >= 1:
                        mulq.extend(norm_mul_items(nq[ib - 1], [0, 1, 2]))
                    if p == 1 and ib >= 1:
                        mulq.extend(norm_mul_items(nq[ib - 1], [3, 4, 5]))
                    if p == 2 and ib >= 1:
                        mulq.extend(norm_mul_items(nq[ib - 1], [6, 7]))
                        # all muls MUST be emitted before any P(ib-1) item
                        # pops (emission order defines dependencies)
                        while mulq:
                            mulq.popleft()()
                        if ib == 3:
                            # hold two P(2) tiles back as PE fill for the
                            # final head-pair's reciprocal chain
                            filler.extend(make_p_items(2)[:6])
                            held = make_p_items(2, drain=True)[6:]
                        else:
                            filler.extend(make_p_items(ib - 1))
            # drain: only the final out-projection remains; all of the last
            # i-block's normalization already ran per-section
            while filler:
                pop_one()
            for _, _, fn in held:
                fn()
            for _, _, fn in make_p_items(3, drain=True):
                fn()


def build():
    nc = bacc.Bacc("TRN2", target_bir_lowering=False, debug=False,
                   num_devices=NCORES)
    io = {
        "x": nc.dram_tensor("x", [S, D], MMD, kind="ExternalInput").ap(),
        "wq": nc.dram_tensor("wq", [D, FG], MMD, kind="ExternalInput").ap(),
        "wk": nc.dram_tensor("wk", [D, FG], MMD, kind="ExternalInput").ap(),
        "wv": nc.dram_tensor("wv", [D, FG], MMD, kind="ExternalInput").ap(),
        "wp": nc.dram_tensor("wp", [FG, D], MMD, kind="ExternalInput").ap(),
        "ident": nc.dram_tensor("ident", [128, 128], MMD,
                                kind="ExternalInput").ap(),
        "out": nc.dram_tensor("out", [S, D], F32, kind="ExternalOutput").ap(),
    }
    with tile.TileContext(nc) as tc:
        _body(tc, io)
    nc.compile()
    return nc


def _host_inputs(x, W_attn, b_attn, W_proj):
    import ml_dtypes

    bf16 = ml_dtypes.bfloat16
    assert not np.any(b_attn), "kernel assumes b_attn == 0 (spec fill: zeros)"
    ident = np.eye(128, dtype=bf16)
    in_maps = []
    for c in range(NCORES):
        b, g = divmod(c, 2)
        in_maps.append({
            "x": np.asarray(x[b], dtype=bf16),
            # fold the 1/sqrt(HD) score scale into wq (exact: * 2^-3)
            "wq": np.asarray(
                W_attn[:, g * FG : (g + 1) * FG] * np.float32(0.125),
                dtype=bf16),
            "wk": np.asarray(
                W_attn[:, D + g * FG : D + (g + 1) * FG], dtype=bf16),
            "wv": np.asarray(
                W_attn[:, 2 * D + g * FG : 2 * D + (g + 1) * FG], dtype=bf16),
            "wp": np.asarray(W_proj[g * FG : (g + 1) * FG, :], dtype=bf16),
            "ident": ident,
        })
    return in_maps


_NC_CACHE = {}


def kernel(x, W_attn, b_attn, W_proj, b_proj, _trace=False):
    x = np.asarray(x)
    W_attn = np.asarray(W_attn)
    b_attn = np.asarray(b_attn)
    W_proj = np.asarray(W_proj)
    b_proj = np.asarray(b_proj)

    if "nc" not in _NC_CACHE:
        _NC_CACHE["nc"] = build()
    nc = _NC_CACHE["nc"]

    in_maps = _host_inputs(x, W_attn, b_attn, W_proj)
    kwargs = {}
    if _trace:
        _install_ntff_hook()
        kwargs = dict(trace=True, trace_cores=[0])
    res = run_bass_kernel_spmd(nc, in_maps, core_ids=list(range(NCORES)),
                               **kwargs)
    y = np.empty((B, S, D), dtype=np.float32)
    for b in range(B):
        y[b] = (res.results[2 * b]["out"] + res.results[2 * b + 1]["out"]
                + b_proj.astype(np.float32))
    if _trace:
        kernel.last_exec_time_ns = res.exec_time_ns
        kernel.last_trace = res.instructions_and_trace
    return y
